# revision 31
# baseline (speedup 1.0000x reference)
"""Trainium2 Bass kernel for CausalSelfAttention (GQA + per-head RMS norm + RoPE).

Sharding: 8 cores = batch(2) x kv-head-group(4). Each core computes, for its
(b, g): qkv projection (its 4 rep q heads + 1 kv head), per-head RMS norm,
RoPE, causal attention, and a partial output projection (its 512 rows of
w_proj). Host sums the 4 partial projections per batch element.

Math notes:
  - Token-level rms_norm(x) commutes out of q/k (they are re-normalized per
    head, and rms_norm is scale-invariant), so only v is scaled by the
    per-token rstd(x). The qkv matmul runs on raw x^T (built via PE
    transposes).
  - Scores are computed transposed (scoresT[s, t]) so exp(scoresT) is
    directly the rhs of the attn@v matmul and the attention output lands as
    aoT[d, t] = the lhsT the output projection needs.
  - Softmax runs without max subtraction: q, k are unit-RMS so scores*scale
    is bounded by ~17 and exp stays in range (bf16 max 3.4e38).
  - Causal masking: diagonal-band score tiles are computed over the full
    chunk width (keeps values bounded), exp'd, then multiplied by a 0/1 mask
    on the affected column range only. attn@v matmuls skip the fully-masked
    column ranges.
  - Denominator: exp tiles are accumulated pairwise on DVE in bf16 (fp32
    internally, one rounding per pair-add), then partition-summed in fp32
    PSUM via an all-ones matmul that also broadcasts the result to all 128
    partitions; the rescale 1/den is applied to aoT before the projection.
  - bf16 is used for every matmul operand (activations and weights) and for
    all wide DVE traffic: the PE streams bf16 at full rate (fp32 would be
    half rate), LDWEIGHTS gets the fast-weight-load path, and DVE element
    ops run in the 2x/4x packed modes. PSUM accumulation stays fp32, as do
    all norm statistics and the softmax denominator, and the output y.
"""

import os

import numpy as np

from concourse import bacc, bass, mybir
from concourse import tile
from concourse.bass_utils import run_bass_kernel_spmd

# Problem shape (hardcoded per contract)
B, T, C = 2, 2048, 2048
N_HEADS, N_KV = 16, 4
HD = C // N_HEADS            # 128
REP = N_HEADS // N_KV        # 4
KV_DIM = N_KV * HD           # 512
P = 128
TT = T // P                  # 16 token tiles
KT = C // P                  # 16 contraction tiles
JQ = REP * HD                # 512 local q cols
JTOT = JQ + 2 * HD           # 768 local qkv cols
TCW = 512                    # attention t-chunk width
NTC = T // TCW               # 4
EPS = 1.1920929e-07
SCALE = 1.0 / float(np.sqrt(HD))

F32 = mybir.dt.float32
BF = mybir.dt.bfloat16
AF = mybir.ActivationFunctionType
MULT = mybir.AluOpType.mult
AXX = mybir.AxisListType.X


def _emit(nc):
    x_d = nc.dram_tensor("xb", [T, C], BF, kind="ExternalInput")
    wqkv_d = nc.dram_tensor("wqkv", [C, JTOT], BF, kind="ExternalInput")
    wproj_d = nc.dram_tensor("wproj", [JQ, C], BF, kind="ExternalInput")
    gain_d = nc.dram_tensor("gain", [1, REP], F32, kind="ExternalInput")
    cos_d = nc.dram_tensor("costab", [T, HD], BF, kind="ExternalInput")
    sin_d = nc.dram_tensor("sintab", [T, HD], BF, kind="ExternalInput")  # [:, :64] = -sin
    mask_d = nc.dram_tensor("mask01", [4, P, TCW], BF, kind="ExternalInput")  # 0 / 1
    id_d = nc.dram_tensor("ident", [P, P], BF, kind="ExternalInput")
    y_d = nc.dram_tensor("y", [T, C], F32, kind="ExternalOutput")

    with tile.TileContext(nc) as tc:
        with tc.tile_pool(name="persist", bufs=1) as pp, \
             tc.tile_pool(name="psum", bufs=1, space="PSUM") as psp:
            # Long-lived bf16 activations (matmul operands)
            qT = pp.tile([P, REP, T], BF, name="qT", tag="qT")
            kTt = pp.tile([P, T], BF, name="kTt", tag="kTt")
            vN = pp.tile([P, TT, HD], BF, name="vN", tag="vN")
            mask_sb = pp.tile([P, 4, TCW], BF, name="mask_sb", tag="mask")
            id_sb = pp.tile([P, P], BF, name="id_sb", tag="ident")
            ones_sb = pp.tile([P, P], BF, name="ones_sb", tag="ones")
            onesf = pp.tile([P, P], F32, name="onesf", tag="onesf")
            eps_t = pp.tile([P, 1], F32, name="eps_t", tag="eps")
            gainb = pp.tile([P, REP], F32, name="gainb", tag="gainb")

            # ---------------- Phase 1: qkv + norms + rope + transposes -------
            with tc.tile_pool(name="ph1", bufs=1) as p1:
                # startup DMAs: first x tiles + small tables land before the
                # big weight tensor so compute starts immediately
                x_tiles = []
                cs_tiles = []
                cos4 = cos_d.ap().rearrange("(tt p) d -> p tt d", p=P)
                sin4 = sin_d.ap().rearrange("(tt p) d -> p tt d", p=P)
                for tt in range(2):
                    x_t = p1.tile([P, C], BF, name=f"x_{tt}", tag="x", bufs=4)
                    nc.sync.dma_start(out=x_t, in_=x_d.ap()[tt * P:(tt + 1) * P, :])
                    x_tiles.append(x_t)
                    cos_t = p1.tile([P, HD], BF, name=f"cos_{tt}", tag="cos",
                                    bufs=2)
                    nc.sync.dma_start(out=cos_t, in_=cos4[:, tt])
                    sin_t = p1.tile([P, HD], BF, name=f"sin_{tt}", tag="sin",
                                    bufs=2)
                    nc.sync.dma_start(out=sin_t, in_=sin4[:, tt])
                    cs_tiles.append((cos_t, sin_t))
                nc.sync.dma_start(out=id_sb, in_=id_d.ap())
                nc.sync.dma_start(out=mask_sb,
                                  in_=mask_d.ap().rearrange("v p t -> p v t"))
                nc.sync.dma_start(out=gainb,
                                  in_=gain_d.ap()[0].partition_broadcast(P))
                nc.vector.memset(onesf, 1.0)
                nc.vector.tensor_copy(ones_sb, onesf)
                nc.vector.memset(eps_t, EPS)

                wqkv_sb = p1.tile([P, KT, JTOT], BF, name="wqkv_sb", tag="wqkv")
                wq4 = wqkv_d.ap().rearrange("(kg kt p) j -> p kg kt j", p=P, kg=4)
                for kg in range(4):
                    for sub in range(2):
                        nc.scalar.dma_start(
                            out=wqkv_sb[:, kg * 4 + sub * 2:kg * 4 + (sub + 1) * 2, :],
                            in_=wq4[:, kg, sub * 2:(sub + 1) * 2])

                H2 = HD // 2

                for tt in range(TT):
                    if tt < 2:
                        x_t = x_tiles[tt]
                    else:
                        x_t = p1.tile([P, C], BF, name=f"x_{tt}", tag="x", bufs=4)
                        nc.sync.dma_start(out=x_t,
                                          in_=x_d.ap()[tt * P:(tt + 1) * P, :])
                    if tt < 2:
                        cos_t, sin_t = cs_tiles[tt]
                    else:
                        cos_t = p1.tile([P, HD], BF, name=f"cos_{tt}", tag="cos",
                                        bufs=2)
                        nc.sync.dma_start(out=cos_t, in_=cos4[:, tt])
                        sin_t = p1.tile([P, HD], BF, name=f"sin_{tt}", tag="sin",
                                        bufs=2)
                        nc.sync.dma_start(out=sin_t, in_=sin4[:, tt])

                    # token rstd (for v): 1/sqrt(mean(x^2)+eps)
                    scrx = p1.tile([P, C], BF, name=f"scrx_{tt}", tag="scr", bufs=2)
                    ssx = p1.tile([P, 1], F32, name=f"ssx_{tt}", tag="ssx", bufs=2)
                    nc.scalar.activation(scrx, x_t, AF.Square, accum_out=ssx)
                    srx = p1.tile([P, 1], F32, name=f"srx_{tt}", tag="srx", bufs=2)
                    nc.scalar.activation(srx, ssx, AF.Sqrt, scale=1.0 / C, bias=eps_t)
                    rstdx = p1.tile([P, 1], F32, name=f"rstdx_{tt}", tag="rstdx", bufs=2)
                    nc.vector.reciprocal(rstdx, srx)

                    # transpose raw x tile -> xT blocks (lhsT for qkv matmul);
                    # all 16 transposes fit one [P,2,512]f32 bank pair as bf16
                    xTt = p1.tile([P, C], BF, name=f"xT_{tt}", tag="xT", bufs=3)
                    tp = psp.tile([P, 2, TCW], F32, name=f"tp_{tt}",
                                  tag="sc", bufs=2)
                    tpv = tp.bitcast(BF)  # [P, 2, 1024]
                    for grp in range(4):
                        bnk, off = grp // 2, (grp % 2) * 512
                        for j in range(4):
                            kt = grp * 4 + j
                            nc.tensor.transpose(
                                tpv[:, bnk, off + j * P:off + (j + 1) * P],
                                x_t[:, kt * P:(kt + 1) * P], id_sb)
                        if grp % 2 == 0:
                            nc.vector.tensor_copy(xTt[:, grp * 512:(grp + 1) * 512],
                                                  tpv[:, bnk, off:off + 512])
                        else:
                            nc.scalar.copy(xTt[:, grp * 512:(grp + 1) * 512],
                                           tpv[:, bnk, off:off + 512])

                    # qkv matmuls: q_ps [P, 512], kv_ps [P, 256]
                    q_ps = psp.tile([P, JQ], F32, name=f"qps_{tt}", tag="acc", bufs=2)
                    kv_big = psp.tile([P, 512], F32, name=f"kvps_{tt}", tag="small",
                                      bufs=2)
                    kv_ps = kv_big[:, 0:2 * HD]
                    for kt in range(KT):
                        lb = xTt[:, kt * P:(kt + 1) * P]
                        nc.tensor.matmul(q_ps, lb, wqkv_sb[:, kt, 0:JQ],
                                         start=(kt == 0), stop=(kt == KT - 1))
                        nc.tensor.matmul(kv_ps, lb, wqkv_sb[:, kt, JQ:JTOT],
                                         start=(kt == 0), stop=(kt == KT - 1))

                    # ---- q: per-head rms norm (x gain) ----
                    sq_q = p1.tile([P, JQ], BF, name=f"sqq_{tt}", tag="sqq", bufs=2)
                    nc.scalar.activation(sq_q, q_ps, AF.Square)
                    ssq4 = p1.tile([P, REP], F32, name=f"ssq4_{tt}", tag="ssq4", bufs=2)
                    nc.vector.reduce_sum(ssq4, sq_q.rearrange("p (h d) -> p h d", h=REP),
                                         axis=AXX)
                    srq = p1.tile([P, REP], F32, name=f"srq_{tt}", tag="srq", bufs=2)
                    nc.scalar.activation(srq, ssq4, AF.Sqrt, scale=1.0 / HD, bias=eps_t)
                    rstdq = p1.tile([P, REP], F32, name=f"rstdq_{tt}", tag="rstdq",
                                    bufs=2)
                    nc.vector.reciprocal(rstdq, srq)
                    rstdqg = p1.tile([P, REP], F32, name=f"rstdqg_{tt}", tag="rstdqg",
                                     bufs=2)
                    nc.vector.tensor_mul(rstdqg, rstdq, gainb)

                    # ---- k: rms norm stats ----
                    sq_k = p1.tile([P, HD], BF, name=f"sqk_{tt}", tag="sqk", bufs=2)
                    nc.scalar.activation(sq_k, kv_ps[:, 0:HD], AF.Square)
                    ssk = p1.tile([P, 1], F32, name=f"ssk_{tt}", tag="ssk", bufs=2)
                    nc.vector.reduce_sum(ssk, sq_k, axis=AXX)
                    srk = p1.tile([P, 1], F32, name=f"srk_{tt}", tag="srk", bufs=2)
                    nc.scalar.activation(srk, ssk, AF.Sqrt, scale=1.0 / HD, bias=eps_t)
                    rstdk = p1.tile([P, 1], F32, name=f"rstdk_{tt}", tag="rstdk",
                                    bufs=2)
                    nc.vector.reciprocal(rstdk, srk)

                    with nc.allow_low_precision(reason="bf16 matmul operands"):
                        # normalized q (bf16): 2 heads on DVE, 2 on ACT
                        qn_t = p1.tile([P, JQ], BF, name=f"qn_{tt}", tag="qn", bufs=2)
                        for h in range(REP):
                            dst = qn_t[:, h * HD:(h + 1) * HD]
                            src = q_ps[:, h * HD:(h + 1) * HD]
                            if h % 2 == 0:
                                nc.vector.tensor_scalar_mul(dst, src,
                                                            rstdqg[:, h:h + 1])
                            else:
                                nc.scalar.mul(dst, src, rstdqg[:, h:h + 1])
                        # rope on q (bf16 DVE)
                        qn3 = qn_t.rearrange("p (h d) -> p h d", h=REP)
                        qf_t = p1.tile([P, JQ], BF, name=f"qf_{tt}", tag="qf", bufs=2)
                        qf3 = qf_t.rearrange("p (h d) -> p h d", h=REP)
                        qB_t = p1.tile([P, JQ], BF, name=f"qB_{tt}", tag="qB", bufs=2)
                        qB3 = qB_t.rearrange("p (h d) -> p h d", h=REP)
                        nc.vector.tensor_mul(qf3, qn3,
                                             cos_t[:, None, :].broadcast_to([P, REP, HD]))
                        nc.vector.tensor_mul(qB3[:, :, 0:H2], qn3[:, :, H2:HD],
                                             sin_t[:, None, 0:H2].broadcast_to([P, REP, H2]))
                        nc.vector.tensor_mul(qB3[:, :, H2:HD], qn3[:, :, 0:H2],
                                             sin_t[:, None, H2:HD].broadcast_to([P, REP, H2]))
                        nc.vector.tensor_add(qf3, qf3, qB3)

                        # k: normalize + rope (bf16)
                        kn_t = p1.tile([P, HD], BF, name=f"kn_{tt}", tag="kn", bufs=2)
                        nc.scalar.mul(kn_t, kv_ps[:, 0:HD], rstdk)
                        kf_t = p1.tile([P, HD], BF, name=f"kf_{tt}", tag="kf", bufs=2)
                        kB_t = p1.tile([P, HD], BF, name=f"kB_{tt}", tag="kB", bufs=2)
                        nc.vector.tensor_mul(kf_t, kn_t, cos_t)
                        nc.vector.tensor_mul(kB_t[:, 0:H2], kn_t[:, H2:HD], sin_t[:, 0:H2])
                        nc.vector.tensor_mul(kB_t[:, H2:HD], kn_t[:, 0:H2], sin_t[:, H2:HD])
                        nc.vector.tensor_add(kf_t, kf_t, kB_t)

                        # v: scale rows by token rstd (ACT, per-partition scale)
                        nc.scalar.mul(vN[:, tt, :], kv_ps[:, HD:2 * HD], rstdx)

                    # roped q/k -> transposed layouts via DMA XBAR (no PE)
                    nc.scalar.dma_start(out=qT[:, :, tt * P:(tt + 1) * P],
                                        in_=qf_t, transpose=True)
                    nc.scalar.dma_start(out=kTt[:, tt * P:(tt + 1) * P],
                                        in_=kf_t, transpose=True)

            # ---------------- Phase 2+3: attention + projection --------------
            with tc.tile_pool(name="ph23", bufs=1) as p2:
                wproj_sb = p2.tile([P, REP, C], BF, name="wproj_sb", tag="wproj")
                wp4 = wproj_d.ap().rearrange("(h p) c -> p h c", p=P)
                for h in range(REP):
                    nc.scalar.dma_start(out=wproj_sb[:, h:h + 1, :],
                                        in_=wp4[:, h:h + 1, :])

                def attend(tci, h, o_ps):
                    """Yields once per s-tile pair; caller interleaves."""
                    nst = 4 * (tci + 1)
                    diag0 = 4 * tci
                    et = p2.tile([P, TT, TCW], BF, name=f"et_{tci}_{h}",
                                 tag="et", bufs=3)
                    denf2 = p2.tile([P, 2, TCW], BF, name=f"dn_{tci}_{h}",
                                    tag="denf", bufs=4)
                    for j in range(nst // 2):
                        sc = psp.tile([P, 2, TCW], F32, name=f"sc_{tci}_{h}_{j}",
                                      tag="sc", bufs=2)
                        for jj in range(2):
                            st = 2 * j + jj
                            nc.tensor.matmul(sc[:, jj, :],
                                             kTt[:, st * P:(st + 1) * P],
                                             qT[:, h, tci * TCW:(tci + 1) * TCW],
                                             start=True, stop=True)
                        with nc.allow_low_precision(reason="bf16 exp tiles"):
                            nc.scalar.activation(et[:, 2 * j:2 * j + 2, :], sc,
                                                 AF.Exp, scale=SCALE)
                            for jj in range(2):
                                st = 2 * j + jj
                                dv = st - diag0
                                if dv >= 0:  # 0/1 mask on the diagonal band
                                    w = P * (dv + 1)
                                    nc.vector.tensor_mul(et[:, st, 0:w],
                                                         et[:, st, 0:w],
                                                         mask_sb[:, dv, 0:w])
                            for jj in range(2):
                                st = 2 * j + jj
                                dv = st - diag0
                                lo = P * dv if dv > 0 else 0
                                nc.tensor.matmul(o_ps[:, lo:TCW], vN[:, st, :],
                                                 et[:, st, lo:TCW],
                                                 start=(st == 0),
                                                 stop=(st == nst - 1))
                            if j == 0:
                                nc.vector.tensor_copy(denf2, et[:, 0:2, :])
                            else:
                                nc.vector.tensor_add(denf2, denf2,
                                                     et[:, 2 * j:2 * j + 2, :])
                        yield
                    # tail: denom partition-sum + broadcast (ones matmul),
                    # reciprocal, rescale
                    rb_ps = psp.tile([P, TCW], F32, name=f"rb_{tci}_{h}",
                                     tag="small", bufs=2)
                    nc.tensor.matmul(rb_ps, ones_sb, denf2[:, 0, :],
                                     start=True, stop=False)
                    nc.tensor.matmul(rb_ps, ones_sb, denf2[:, 1, :],
                                     start=False, stop=True)
                    rb = p2.tile([P, TCW], F32, name=f"rbs_{tci}_{h}",
                                 tag="rb", bufs=4)
                    nc.vector.reciprocal_approx_fast(rb, rb_ps)
                    aot = p2.tile([P, TCW], BF, name=f"ao_{tci}_{h}",
                                  tag="ao", bufs=8)
                    with nc.allow_low_precision(reason="bf16 matmul operand"):
                        nc.vector.tensor_mul(aot, o_ps, rb)
                    ao_tiles[(tci, h)] = aot
                    yield

                def proj_steps(tci):
                    """Projection for chunk tci as small emit-steps (4 MMs each)."""
                    for ttl in range(4):
                        yt = p2.tile([P, C], F32, name=f"y_{tci}_{ttl}", tag="y", bufs=2)
                        for ncs in range(4):
                            def step(tci=tci, ttl=ttl, ncs=ncs, yt=yt):
                                y_ps = psp.tile([P, 512], F32,
                                                name=f"yps_{tci}_{ttl}_{ncs}",
                                                tag="small", bufs=2)
                                for h in range(REP):
                                    nc.tensor.matmul(
                                        y_ps,
                                        ao_tiles[(tci, h)][:, ttl * P:(ttl + 1) * P],
                                        wproj_sb[:, h, ncs * 512:(ncs + 1) * 512],
                                        start=(h == 0), stop=(h == REP - 1))
                                if ncs % 2 == 0:
                                    nc.vector.tensor_copy(
                                        yt[:, ncs * 512:(ncs + 1) * 512], y_ps)
                                else:
                                    nc.scalar.copy(
                                        yt[:, ncs * 512:(ncs + 1) * 512], y_ps)
                            yield step
                        def dma_step(tci=tci, ttl=ttl, yt=yt):
                            row = (tci * 4 + ttl) * P
                            nc.sync.dma_start(out=y_d.ap()[row:row + P, :], in_=yt)
                        yield dma_step

                ao_tiles = {}
                from collections import deque
                todo = deque((tci, h) for tci in range(NTC) for h in range(REP))
                active = []
                steps_q = deque()
                while todo or active:
                    # keep two head-generators in flight, rolling across
                    # head-pair and chunk boundaries
                    while len(active) < 2 and todo:
                        tci, h = todo.popleft()
                        o = psp.tile([P, TCW], F32, name=f"ops_{tci}_{h}",
                                     tag="acc", bufs=2)
                        active.append((attend(tci, h, o), tci, h))
                    for rec in list(active):
                        try:
                            next(rec[0])
                        except StopIteration:
                            active.remove(rec)
                            if rec[2] == REP - 1:
                                steps_q.extend(proj_steps(rec[1]))
                    # weave projection steps of completed chunks
                    nw = 2 if len(active) < 2 else 1
                    for _ in range(nw):
                        if steps_q:
                            steps_q.popleft()()
                while steps_q:
                    steps_q.popleft()()

    return nc


_NC_CACHE = {}
LAST_RESULT = None


def _get_nc():
    if "nc" not in _NC_CACHE:
        nc = bacc.Bacc("TRN2", target_bir_lowering=False, debug=False)
        _emit(nc)
        nc.compile()
        _NC_CACHE["nc"] = nc
    return _NC_CACHE["nc"]


def _host_tables():
    import ml_dtypes
    bf16 = ml_dtypes.bfloat16
    inv_freq = 1.0 / (10000.0 ** (np.arange(0, HD, 2, dtype=np.float64) / HD))
    t = np.arange(T, dtype=np.float64)
    freqs = np.outer(t, inv_freq)                      # [T, 64]
    emb = np.concatenate([freqs, freqs], axis=-1)      # [T, 128]
    cos = np.cos(emb).astype(bf16)
    sin = np.sin(emb).astype(np.float64)
    sin[:, :HD // 2] *= -1.0                           # first half gets -sin
    sin = sin.astype(bf16)
    # 0/1 mask for diagonal-band score tiles, scoresT layout:
    # mask[v][s, t] = 1 if (128v+s <= t) else 0
    mask = np.zeros((4, P, TCW), dtype=bf16)
    s = np.arange(P)[:, None]
    tcol = np.arange(TCW)[None, :]
    for v in range(4):
        mask[v] = (v * P + s <= tcol).astype(bf16)
    ident = np.eye(P, dtype=bf16)
    return cos, sin, mask, ident


def kernel(x, w_qkv, w_proj, q_gain):
    global LAST_RESULT
    import ml_dtypes
    bf16 = ml_dtypes.bfloat16
    x = np.asarray(x, dtype=np.float32).astype(bf16)
    w_qkv = np.asarray(w_qkv, dtype=np.float32).astype(bf16)
    w_proj = np.asarray(w_proj, dtype=np.float32).astype(bf16)
    q_gain = np.asarray(q_gain, dtype=np.float32)

    cos, sin_signed, mask, ident = _host_tables()
    nc = _get_nc()

    in_maps = []
    for r in range(8):
        b, g = r // 4, r % 4
        wq = w_qkv[:, g * JQ:(g + 1) * JQ]
        wk = w_qkv[:, C + g * HD:C + (g + 1) * HD]
        wv = w_qkv[:, C + KV_DIM + g * HD:C + KV_DIM + (g + 1) * HD]
        in_maps.append({
            "xb": np.ascontiguousarray(x[b]),
            "wqkv": np.ascontiguousarray(np.concatenate([wq, wk, wv], axis=1)),
            "wproj": np.ascontiguousarray(w_proj[g * JQ:(g + 1) * JQ, :]),
            "gain": np.ascontiguousarray(q_gain[g * REP:(g + 1) * REP].reshape(1, REP)),
            "costab": cos,
            "sintab": sin_signed,
            "mask01": mask,
            "ident": ident,
        })

    trace = os.environ.get("KERNEL_TRACE") == "1"
    if trace:
        try:
            import antenv.axon_hooks  # noqa: F401
        except ImportError:
            trace = False
    res = run_bass_kernel_spmd(nc, in_maps, core_ids=list(range(8)), trace=trace)
    LAST_RESULT = res

    out = np.zeros((B, T, C), dtype=np.float32)
    for r in range(8):
        b = r // 4
        out[b] += res.results[r]["y"]
    return out


# revision 32
# speedup vs baseline: 1.1867x; 1.1867x over previous
"""Trainium2 Bass kernel for CausalSelfAttention (GQA + per-head RMS norm + RoPE).

Sharding: 8 cores = batch(2) x kv-head-group(4). Each core computes, for its
(b, g): qkv projection (its 4 rep q heads + 1 kv head), per-head RMS norm,
RoPE, causal attention, and a partial output projection (its 512 rows of
w_proj). Host sums the 4 partial projections per batch element.

Math notes:
  - Token-level rms_norm(x) commutes out of q/k (they are re-normalized per
    head, and rms_norm is scale-invariant), so only v is scaled by the
    per-token rstd(x). The qkv matmul runs on raw x^T (built via PE
    transposes).
  - Scores are computed transposed (scoresT[s, t]) so exp(scoresT) is
    directly the rhs of the attn@v matmul and the attention output lands as
    aoT[d, t] = the lhsT the output projection needs.
  - Softmax runs without max subtraction: q, k are unit-RMS so scores*scale
    is bounded by ~17 and exp stays in range (bf16 max 3.4e38).
  - Causal masking: diagonal-band score tiles are computed over the full
    chunk width (keeps values bounded), exp'd, then multiplied by a 0/1 mask
    on the affected column range only. attn@v matmuls skip the fully-masked
    column ranges.
  - Denominator: exp tiles are accumulated pairwise on DVE in bf16 (fp32
    internally, one rounding per pair-add), then partition-summed in fp32
    PSUM via an all-ones matmul that also broadcasts the result to all 128
    partitions; the rescale 1/den is applied to aoT before the projection.
  - bf16 is used for every matmul operand (activations and weights) and for
    all wide DVE traffic: the PE streams bf16 at full rate (fp32 would be
    half rate), LDWEIGHTS gets the fast-weight-load path, and DVE element
    ops run in the 2x/4x packed modes. PSUM accumulation stays fp32, as do
    all norm statistics and the softmax denominator, and the output y.
"""

import os

import numpy as np

from concourse import bacc, bass, mybir
from concourse import tile
from concourse.bass_utils import run_bass_kernel_spmd

# Problem shape (hardcoded per contract)
B, T, C = 2, 2048, 2048
N_HEADS, N_KV = 16, 4
HD = C // N_HEADS            # 128
REP = N_HEADS // N_KV        # 4
KV_DIM = N_KV * HD           # 512
P = 128
TT = T // P                  # 16 token tiles
KT = C // P                  # 16 contraction tiles
JQ = REP * HD                # 512 local q cols
JTOT = JQ + 2 * HD           # 768 local qkv cols
TCW = 512                    # attention t-chunk width
NTC = T // TCW               # 4
EPS = 1.1920929e-07
SCALE = 1.0 / float(np.sqrt(HD))

F32 = mybir.dt.float32
BF = mybir.dt.bfloat16
AF = mybir.ActivationFunctionType
MULT = mybir.AluOpType.mult
AXX = mybir.AxisListType.X


def _emit(nc):
    x_d = nc.dram_tensor("xb", [T, C], BF, kind="ExternalInput")
    wqkv_d = nc.dram_tensor("wqkv", [C, JTOT], BF, kind="ExternalInput")
    wproj_d = nc.dram_tensor("wproj", [JQ, C], BF, kind="ExternalInput")
    gain_d = nc.dram_tensor("gain", [1, REP], F32, kind="ExternalInput")
    cos_d = nc.dram_tensor("costab", [T, HD], BF, kind="ExternalInput")
    sin_d = nc.dram_tensor("sintab", [T, HD], BF, kind="ExternalInput")  # [:, :64] = -sin
    mask_d = nc.dram_tensor("mask01", [4, P, TCW], BF, kind="ExternalInput")  # 0 / 1
    id_d = nc.dram_tensor("ident", [P, P], BF, kind="ExternalInput")
    y_d = nc.dram_tensor("y", [T, C], F32, kind="ExternalOutput")

    with tile.TileContext(nc) as tc:
        with tc.tile_pool(name="persist", bufs=1) as pp, \
             tc.tile_pool(name="psum", bufs=1, space="PSUM") as psp:
            # Long-lived bf16 activations (matmul operands)
            qT = pp.tile([P, REP, T], BF, name="qT", tag="qT")
            kTt = pp.tile([P, T], BF, name="kTt", tag="kTt")
            vN = pp.tile([P, TT, HD], BF, name="vN", tag="vN")
            mask_sb = pp.tile([P, 4, TCW], BF, name="mask_sb", tag="mask")
            id_sb = pp.tile([P, P], BF, name="id_sb", tag="ident")
            ones_sb = pp.tile([P, P], BF, name="ones_sb", tag="ones")
            onesf = pp.tile([P, P], F32, name="onesf", tag="onesf")
            eps_t = pp.tile([P, 1], F32, name="eps_t", tag="eps")
            gainb = pp.tile([P, REP], F32, name="gainb", tag="gainb")

            # ---------------- Phase 1: qkv + norms + rope + transposes -------
            with tc.tile_pool(name="ph1", bufs=1) as p1:
                # startup DMAs: first x tiles + small tables land before the
                # big weight tensor so compute starts immediately
                x_tiles = []
                cs_tiles = []
                cos4 = cos_d.ap().rearrange("(tt p) d -> p tt d", p=P)
                sin4 = sin_d.ap().rearrange("(tt p) d -> p tt d", p=P)
                for tt in range(2):
                    x_t = p1.tile([P, C], BF, name=f"x_{tt}", tag="x", bufs=4)
                    nc.sync.dma_start(out=x_t, in_=x_d.ap()[tt * P:(tt + 1) * P, :])
                    x_tiles.append(x_t)
                    cos_t = p1.tile([P, HD], BF, name=f"cos_{tt}", tag="cos",
                                    bufs=2)
                    nc.sync.dma_start(out=cos_t, in_=cos4[:, tt])
                    sin_t = p1.tile([P, HD], BF, name=f"sin_{tt}", tag="sin",
                                    bufs=2)
                    nc.sync.dma_start(out=sin_t, in_=sin4[:, tt])
                    cs_tiles.append((cos_t, sin_t))
                nc.sync.dma_start(out=id_sb, in_=id_d.ap())
                nc.sync.dma_start(out=mask_sb,
                                  in_=mask_d.ap().rearrange("v p t -> p v t"))
                nc.sync.dma_start(out=gainb,
                                  in_=gain_d.ap()[0].partition_broadcast(P))
                nc.vector.memset(onesf, 1.0)
                nc.vector.tensor_copy(ones_sb, onesf)
                nc.vector.memset(eps_t, EPS)

                wqkv_sb = p1.tile([P, KT, JTOT], BF, name="wqkv_sb", tag="wqkv")
                wq4 = wqkv_d.ap().rearrange("(kg kt p) j -> p kg kt j", p=P, kg=4)
                for kg in range(4):
                    for sub in range(2):
                        nc.scalar.dma_start(
                            out=wqkv_sb[:, kg * 4 + sub * 2:kg * 4 + (sub + 1) * 2, :],
                            in_=wq4[:, kg, sub * 2:(sub + 1) * 2])

                H2 = HD // 2

                for tt in range(TT):
                    if tt < 2:
                        x_t = x_tiles[tt]
                    else:
                        x_t = p1.tile([P, C], BF, name=f"x_{tt}", tag="x", bufs=4)
                        nc.sync.dma_start(out=x_t,
                                          in_=x_d.ap()[tt * P:(tt + 1) * P, :])
                    if tt < 2:
                        cos_t, sin_t = cs_tiles[tt]
                    else:
                        cos_t = p1.tile([P, HD], BF, name=f"cos_{tt}", tag="cos",
                                        bufs=2)
                        nc.sync.dma_start(out=cos_t, in_=cos4[:, tt])
                        sin_t = p1.tile([P, HD], BF, name=f"sin_{tt}", tag="sin",
                                        bufs=2)
                        nc.sync.dma_start(out=sin_t, in_=sin4[:, tt])

                    # token rstd (for v): 1/sqrt(mean(x^2)+eps)
                    scrx = p1.tile([P, C], BF, name=f"scrx_{tt}", tag="scr", bufs=2)
                    ssx = p1.tile([P, 1], F32, name=f"ssx_{tt}", tag="ssx", bufs=2)
                    nc.scalar.activation(scrx, x_t, AF.Square, accum_out=ssx)
                    srx = p1.tile([P, 1], F32, name=f"srx_{tt}", tag="srx", bufs=2)
                    nc.scalar.activation(srx, ssx, AF.Sqrt, scale=1.0 / C, bias=eps_t)
                    rstdx = p1.tile([P, 1], F32, name=f"rstdx_{tt}", tag="rstdx", bufs=2)
                    nc.vector.reciprocal(rstdx, srx)

                    # transpose raw x tile -> xT blocks (lhsT for qkv matmul);
                    # all 16 transposes fit one [P,2,512]f32 bank pair as bf16
                    xTt = p1.tile([P, C], BF, name=f"xT_{tt}", tag="xT", bufs=3)
                    tp = psp.tile([P, 2, TCW], F32, name=f"tp_{tt}",
                                  tag="sc", bufs=2)
                    tpv = tp.bitcast(BF)  # [P, 2, 1024]
                    for grp in range(4):
                        bnk, off = grp // 2, (grp % 2) * 512
                        for j in range(4):
                            kt = grp * 4 + j
                            nc.tensor.transpose(
                                tpv[:, bnk, off + j * P:off + (j + 1) * P],
                                x_t[:, kt * P:(kt + 1) * P], id_sb)
                        if grp % 2 == 0:
                            nc.vector.tensor_copy(xTt[:, grp * 512:(grp + 1) * 512],
                                                  tpv[:, bnk, off:off + 512])
                        else:
                            nc.scalar.copy(xTt[:, grp * 512:(grp + 1) * 512],
                                           tpv[:, bnk, off:off + 512])

                    # qkv matmuls: q_ps [P, 512], kv_ps [P, 256]
                    q_ps = psp.tile([P, JQ], F32, name=f"qps_{tt}", tag="acc", bufs=2)
                    kv_big = psp.tile([P, 512], F32, name=f"kvps_{tt}", tag="small",
                                      bufs=2)
                    kv_ps = kv_big[:, 0:2 * HD]
                    for kt in range(KT):
                        lb = xTt[:, kt * P:(kt + 1) * P]
                        nc.tensor.matmul(q_ps, lb, wqkv_sb[:, kt, 0:JQ],
                                         start=(kt == 0), stop=(kt == KT - 1))
                        nc.tensor.matmul(kv_ps, lb, wqkv_sb[:, kt, JQ:JTOT],
                                         start=(kt == 0), stop=(kt == KT - 1))

                    # ---- q: per-head rms norm (x gain) ----
                    sq_q = p1.tile([P, JQ], BF, name=f"sqq_{tt}", tag="sqq", bufs=2)
                    nc.scalar.activation(sq_q, q_ps, AF.Square)
                    ssq4 = p1.tile([P, REP], F32, name=f"ssq4_{tt}", tag="ssq4", bufs=2)
                    nc.vector.reduce_sum(ssq4, sq_q.rearrange("p (h d) -> p h d", h=REP),
                                         axis=AXX)
                    srq = p1.tile([P, REP], F32, name=f"srq_{tt}", tag="srq", bufs=2)
                    nc.scalar.activation(srq, ssq4, AF.Sqrt, scale=1.0 / HD, bias=eps_t)
                    rstdq = p1.tile([P, REP], F32, name=f"rstdq_{tt}", tag="rstdq",
                                    bufs=2)
                    nc.vector.reciprocal(rstdq, srq)
                    rstdqg = p1.tile([P, REP], F32, name=f"rstdqg_{tt}", tag="rstdqg",
                                     bufs=2)
                    nc.vector.tensor_mul(rstdqg, rstdq, gainb)

                    # ---- k: rms norm stats ----
                    sq_k = p1.tile([P, HD], BF, name=f"sqk_{tt}", tag="sqk", bufs=2)
                    nc.scalar.activation(sq_k, kv_ps[:, 0:HD], AF.Square)
                    ssk = p1.tile([P, 1], F32, name=f"ssk_{tt}", tag="ssk", bufs=2)
                    nc.vector.reduce_sum(ssk, sq_k, axis=AXX)
                    srk = p1.tile([P, 1], F32, name=f"srk_{tt}", tag="srk", bufs=2)
                    nc.scalar.activation(srk, ssk, AF.Sqrt, scale=1.0 / HD, bias=eps_t)
                    rstdk = p1.tile([P, 1], F32, name=f"rstdk_{tt}", tag="rstdk",
                                    bufs=2)
                    nc.vector.reciprocal(rstdk, srk)

                    with nc.allow_low_precision(reason="bf16 matmul operands"):
                        # normalized q (bf16): 2 heads on DVE, 2 on ACT
                        qn_t = p1.tile([P, JQ], BF, name=f"qn_{tt}", tag="qn", bufs=2)
                        for h in range(REP):
                            dst = qn_t[:, h * HD:(h + 1) * HD]
                            src = q_ps[:, h * HD:(h + 1) * HD]
                            if h % 2 == 0:
                                nc.vector.tensor_scalar_mul(dst, src,
                                                            rstdqg[:, h:h + 1])
                            else:
                                nc.scalar.mul(dst, src, rstdqg[:, h:h + 1])
                        # rope on q (bf16 DVE)
                        qn3 = qn_t.rearrange("p (h d) -> p h d", h=REP)
                        qf_t = p1.tile([P, JQ], BF, name=f"qf_{tt}", tag="qf", bufs=2)
                        qf3 = qf_t.rearrange("p (h d) -> p h d", h=REP)
                        qB_t = p1.tile([P, JQ], BF, name=f"qB_{tt}", tag="qB", bufs=2)
                        qB3 = qB_t.rearrange("p (h d) -> p h d", h=REP)
                        nc.vector.tensor_mul(qf3, qn3,
                                             cos_t[:, None, :].broadcast_to([P, REP, HD]))
                        nc.vector.tensor_mul(qB3[:, :, 0:H2], qn3[:, :, H2:HD],
                                             sin_t[:, None, 0:H2].broadcast_to([P, REP, H2]))
                        nc.vector.tensor_mul(qB3[:, :, H2:HD], qn3[:, :, 0:H2],
                                             sin_t[:, None, H2:HD].broadcast_to([P, REP, H2]))
                        nc.vector.tensor_add(qf3, qf3, qB3)

                        # k: normalize + rope (bf16)
                        kn_t = p1.tile([P, HD], BF, name=f"kn_{tt}", tag="kn", bufs=2)
                        nc.scalar.mul(kn_t, kv_ps[:, 0:HD], rstdk)
                        kf_t = p1.tile([P, HD], BF, name=f"kf_{tt}", tag="kf", bufs=2)
                        kB_t = p1.tile([P, HD], BF, name=f"kB_{tt}", tag="kB", bufs=2)
                        nc.vector.tensor_mul(kf_t, kn_t, cos_t)
                        nc.vector.tensor_mul(kB_t[:, 0:H2], kn_t[:, H2:HD], sin_t[:, 0:H2])
                        nc.vector.tensor_mul(kB_t[:, H2:HD], kn_t[:, 0:H2], sin_t[:, H2:HD])
                        nc.vector.tensor_add(kf_t, kf_t, kB_t)

                        # v: scale rows by token rstd (ACT, per-partition scale)
                        nc.scalar.mul(vN[:, tt, :], kv_ps[:, HD:2 * HD], rstdx)

                    # roped q/k -> transposed layouts via DMA XBAR (no PE)
                    nc.scalar.dma_start(out=qT[:, :, tt * P:(tt + 1) * P],
                                        in_=qf_t, transpose=True)
                    nc.scalar.dma_start(out=kTt[:, tt * P:(tt + 1) * P],
                                        in_=kf_t, transpose=True)

            # ---------------- Phase 2+3: attention + projection --------------
            with tc.tile_pool(name="ph23", bufs=1) as p2:
                wproj_sb = p2.tile([P, REP, C], BF, name="wproj_sb", tag="wproj")
                wp4 = wproj_d.ap().rearrange("(h p) c -> p h c", p=P)
                for h in range(REP):
                    nc.scalar.dma_start(out=wproj_sb[:, h:h + 1, :],
                                        in_=wp4[:, h:h + 1, :])

                def attend(tci, h, o_ps):
                    """Yields once per s-tile pair; caller interleaves."""
                    nst = 4 * (tci + 1)
                    diag0 = 4 * tci
                    et = p2.tile([P, TT, TCW], BF, name=f"et_{tci}_{h}",
                                 tag="et", bufs=3)
                    denf2 = p2.tile([P, 2, TCW], BF, name=f"dn_{tci}_{h}",
                                    tag="denf", bufs=4)
                    for j in range(nst // 2):
                        sc = psp.tile([P, 2, TCW], F32, name=f"sc_{tci}_{h}_{j}",
                                      tag="sc", bufs=2)
                        for jj in range(2):
                            st = 2 * j + jj
                            nc.tensor.matmul(sc[:, jj, :],
                                             kTt[:, st * P:(st + 1) * P],
                                             qT[:, h, tci * TCW:(tci + 1) * TCW],
                                             start=True, stop=True)
                        with nc.allow_low_precision(reason="bf16 exp tiles"):
                            nc.scalar.activation(et[:, 2 * j:2 * j + 2, :], sc,
                                                 AF.Exp, scale=SCALE)
                            for jj in range(2):
                                st = 2 * j + jj
                                dv = st - diag0
                                if dv >= 0:  # 0/1 mask on the diagonal band
                                    w = P * (dv + 1)
                                    nc.vector.tensor_mul(et[:, st, 0:w],
                                                         et[:, st, 0:w],
                                                         mask_sb[:, dv, 0:w])
                            for jj in range(2):
                                st = 2 * j + jj
                                dv = st - diag0
                                lo = P * dv if dv > 0 else 0
                                nc.tensor.matmul(o_ps[:, lo:TCW], vN[:, st, :],
                                                 et[:, st, lo:TCW],
                                                 start=(st == 0),
                                                 stop=(st == nst - 1))
                            if j == 0:
                                nc.vector.tensor_copy(denf2, et[:, 0:2, :])
                            else:
                                nc.vector.tensor_add(denf2, denf2,
                                                     et[:, 2 * j:2 * j + 2, :])
                        yield
                    # tail: denom partition-sum + broadcast (ones matmul),
                    # reciprocal, rescale
                    rb_ps = psp.tile([P, TCW], F32, name=f"rb_{tci}_{h}",
                                     tag="small", bufs=2)
                    nc.tensor.matmul(rb_ps, ones_sb, denf2[:, 0, :],
                                     start=True, stop=False)
                    nc.tensor.matmul(rb_ps, ones_sb, denf2[:, 1, :],
                                     start=False, stop=True)
                    rb = p2.tile([P, TCW], F32, name=f"rbs_{tci}_{h}",
                                 tag="rb", bufs=4)
                    nc.vector.reciprocal_approx_fast(rb, rb_ps)
                    aot = p2.tile([P, TCW], BF, name=f"ao_{tci}_{h}",
                                  tag="ao", bufs=8)
                    with nc.allow_low_precision(reason="bf16 matmul operand"):
                        nc.vector.tensor_mul(aot, o_ps, rb)
                    ao_tiles[(tci, h)] = aot
                    yield

                def proj_steps(tci):
                    """Projection for chunk tci as small emit-steps (4 MMs each)."""
                    for ttl in range(4):
                        yt = p2.tile([P, C], F32, name=f"y_{tci}_{ttl}", tag="y", bufs=2)
                        for ncs in range(4):
                            def step(tci=tci, ttl=ttl, ncs=ncs, yt=yt):
                                y_ps = psp.tile([P, 512], F32,
                                                name=f"yps_{tci}_{ttl}_{ncs}",
                                                tag="small", bufs=2)
                                for h in range(REP):
                                    nc.tensor.matmul(
                                        y_ps,
                                        ao_tiles[(tci, h)][:, ttl * P:(ttl + 1) * P],
                                        wproj_sb[:, h, ncs * 512:(ncs + 1) * 512],
                                        start=(h == 0), stop=(h == REP - 1))
                                if ncs % 2 == 0:
                                    nc.vector.tensor_copy(
                                        yt[:, ncs * 512:(ncs + 1) * 512], y_ps)
                                else:
                                    nc.scalar.copy(
                                        yt[:, ncs * 512:(ncs + 1) * 512], y_ps)
                            yield step
                        def dma_step(tci=tci, ttl=ttl, yt=yt):
                            row = (tci * 4 + ttl) * P
                            nc.sync.dma_start(out=y_d.ap()[row:row + P, :], in_=yt)
                        yield dma_step

                ao_tiles = {}
                pending = iter(())
                for tci in range(NTC):
                    for hp in (0, 2):
                        o0 = psp.tile([P, TCW], F32, name=f"ops_{tci}_{hp}",
                                      tag="acc", bufs=2)
                        o1 = psp.tile([P, TCW], F32, name=f"ops_{tci}_{hp + 1}",
                                      tag="acc", bufs=2)
                        g0 = attend(tci, hp, o0)
                        g1 = attend(tci, hp + 1, o1)
                        alive = True
                        while alive:
                            alive = False
                            for g in (g0, g1):
                                try:
                                    next(g)
                                    alive = True
                                except StopIteration:
                                    pass
                            # weave projection steps of the previous chunk
                            s = next(pending, None)
                            if s is not None:
                                s()
                    # flush any remaining projection steps of the previous chunk
                    for s in pending:
                        s()
                    pending = iter(list(proj_steps(tci)))
                for s in pending:
                    s()

    return nc


_NC_CACHE = {}
LAST_RESULT = None


def _get_nc():
    if "nc" not in _NC_CACHE:
        nc = bacc.Bacc("TRN2", target_bir_lowering=False, debug=False)
        _emit(nc)
        nc.compile()
        _NC_CACHE["nc"] = nc
    return _NC_CACHE["nc"]


def _host_tables():
    import ml_dtypes
    bf16 = ml_dtypes.bfloat16
    inv_freq = 1.0 / (10000.0 ** (np.arange(0, HD, 2, dtype=np.float64) / HD))
    t = np.arange(T, dtype=np.float64)
    freqs = np.outer(t, inv_freq)                      # [T, 64]
    emb = np.concatenate([freqs, freqs], axis=-1)      # [T, 128]
    cos = np.cos(emb).astype(bf16)
    sin = np.sin(emb).astype(np.float64)
    sin[:, :HD // 2] *= -1.0                           # first half gets -sin
    sin = sin.astype(bf16)
    # 0/1 mask for diagonal-band score tiles, scoresT layout:
    # mask[v][s, t] = 1 if (128v+s <= t) else 0
    mask = np.zeros((4, P, TCW), dtype=bf16)
    s = np.arange(P)[:, None]
    tcol = np.arange(TCW)[None, :]
    for v in range(4):
        mask[v] = (v * P + s <= tcol).astype(bf16)
    ident = np.eye(P, dtype=bf16)
    return cos, sin, mask, ident


def kernel(x, w_qkv, w_proj, q_gain):
    global LAST_RESULT
    import ml_dtypes
    bf16 = ml_dtypes.bfloat16
    x = np.asarray(x, dtype=np.float32).astype(bf16)
    w_qkv = np.asarray(w_qkv, dtype=np.float32).astype(bf16)
    w_proj = np.asarray(w_proj, dtype=np.float32).astype(bf16)
    q_gain = np.asarray(q_gain, dtype=np.float32)

    cos, sin_signed, mask, ident = _host_tables()
    nc = _get_nc()

    in_maps = []
    for r in range(8):
        b, g = r // 4, r % 4
        wq = w_qkv[:, g * JQ:(g + 1) * JQ]
        wk = w_qkv[:, C + g * HD:C + (g + 1) * HD]
        wv = w_qkv[:, C + KV_DIM + g * HD:C + KV_DIM + (g + 1) * HD]
        in_maps.append({
            "xb": np.ascontiguousarray(x[b]),
            "wqkv": np.ascontiguousarray(np.concatenate([wq, wk, wv], axis=1)),
            "wproj": np.ascontiguousarray(w_proj[g * JQ:(g + 1) * JQ, :]),
            "gain": np.ascontiguousarray(q_gain[g * REP:(g + 1) * REP].reshape(1, REP)),
            "costab": cos,
            "sintab": sin_signed,
            "mask01": mask,
            "ident": ident,
        })

    trace = os.environ.get("KERNEL_TRACE") == "1"
    if trace:
        try:
            import antenv.axon_hooks  # noqa: F401
        except ImportError:
            trace = False
    res = run_bass_kernel_spmd(nc, in_maps, core_ids=list(range(8)), trace=trace)
    LAST_RESULT = res

    out = np.zeros((B, T, C), dtype=np.float32)
    for r in range(8):
        b = r // 4
        out[b] += res.results[r]["y"]
    return out
